# revision 57
# baseline (speedup 1.0000x reference)
"""Multi-head causal attention (B=2, S=2048, D=1024, H=16) on 8 TRN2 cores.

Sharding: core = (batch b = core//4, head-group g = core%4). Each core
computes 4 heads of one batch end-to-end (QKV projections for its head
slice, causal attention, its partial contribution to the output
projection). Host sums the 4 partial outputs per batch and adds the bias.

Device algorithm (per core), all matmuls in bf16 with f32 PSUM accum:
  qT/kT [dloc=256, S] = Wslice @ x.T   (x.T host-tiled to [IT,NCH,128,512]
                                        so every DMA tile is contiguous)
  V     [S, dloc]     (+ ones column per head for the softmax denominator)
  attention runs per (q-chunk of 512, head-PAIR):
    sT[k,q] for both heads of the pair -> one 2-bank PSUM tile
    attnT = exp(sT * 1/8) in ONE strided ScalarE op per k-tile pair
            (causal: k-tiles above the diagonal skipped, diagonal tiles
             use a q-subrange plus a 0/1 mask multiply)
    per head: AT_aug [65, q] = sum_k V_aug.T @ attnT          -> PSUM
              row 64 = softmax denominator l; AT = AT * bcast(1/l)
  out_partial [S, 1024] = AT.T-free matmul with the Wo slice, bf16 out.
Scheduling: each chunk's independent PE work (projections for upcoming
chunks, Wo for finished chunks) is emitted BETWEEN the scores and the
AV matmuls of each head pair, so the in-order PE queue has dense work
in front of the exp-gated AV chain. Inputs arrive as one 1MB DMA per
(tensor, chunk) on the sync ring; each weight rides its own engine
queue so nothing serializes at startup; output stores (bf16) ride the
GpSimd SWDGE ring.

The device kernel assumes the causal (lower-triangular) mask the
reference constructs; kernel() verifies that and falls back to an exact
numpy implementation for any other mask.
"""

import numpy as np
import ml_dtypes

D_MODEL = 1024
NUM_HEADS = 16
HEAD_DIM = 64
B = 2
S = 2048
N_CORES = 8
GROUPS = 4                 # head-groups (cores per batch)
HPC = NUM_HEADS // GROUPS  # 4 heads per core
DLOC = HPC * HEAD_DIM      # 256 local projection dims
P = 128
SCH = 512                  # q/s chunk
NCH = S // SCH             # 4
KT = S // P                # 16 k-tiles
IT = D_MODEL // P          # 8 contraction tiles
MB = DLOC // P             # 2 m-blocks

_CACHE = {}


def _build():
    import concourse.bass as bass
    import concourse.tile as tile
    from concourse import bacc, mybir

    F32 = mybir.dt.float32
    BF16 = mybir.dt.bfloat16

    nc = bacc.Bacc("TRN2", target_bir_lowering=False, debug=False,
                   num_devices=N_CORES)

    # inputs host-tiled: [IT, NCH, 128, 512] so each (r, c) tile is one
    # contiguous 128KB block; a chunk is one strided 1MB DMA
    xq = nc.dram_tensor("xq_t", [IT, NCH, P, SCH], BF16, kind="ExternalInput")
    xk = nc.dram_tensor("xk_t", [IT, NCH, P, SCH], BF16, kind="ExternalInput")
    xv = nc.dram_tensor("xv_t", [IT, NCH, P, SCH], BF16, kind="ExternalInput")
    wq = nc.dram_tensor("wq_t", [D_MODEL, DLOC], BF16, kind="ExternalInput")
    wk = nc.dram_tensor("wk_t", [D_MODEL, DLOC], BF16, kind="ExternalInput")
    wv = nc.dram_tensor("wv_t", [D_MODEL, DLOC], BF16, kind="ExternalInput")
    wo = nc.dram_tensor("wo_t", [DLOC, D_MODEL], BF16, kind="ExternalInput")
    outp = nc.dram_tensor("outp", [S, D_MODEL], BF16, kind="ExternalOutput")

    Exp = mybir.ActivationFunctionType.Exp

    with tile.TileContext(nc) as tc:
        with (
            tc.tile_pool(name="const", bufs=1) as constp,
            tc.tile_pool(name="persist", bufs=1) as pers,
            tc.tile_pool(name="inp", bufs=5) as inp,
            tc.tile_pool(name="attn", bufs=28) as attnp,
            tc.tile_pool(name="small", bufs=4) as small,
            tc.tile_pool(name="ostage", bufs=8) as ostage,
            tc.tile_pool(name="psA", bufs=2, space="PSUM") as psA,
            tc.tile_pool(name="psS", bufs=2, space="PSUM") as psS,
            tc.tile_pool(name="psO", bufs=2, space="PSUM") as psO,
        ):
            # ---- constants / persistent tensors ----
            wq_sb = constp.tile([P, IT, DLOC], BF16)
            wk_sb = constp.tile([P, IT, DLOC], BF16)
            wv_sb = constp.tile([P, IT, DLOC], BF16)
            wo_sb = constp.tile([P, MB, D_MODEL], BF16)
            mk_sb = constp.tile([P, 4 * SCH], BF16)

            qT_sb = pers.tile([P, MB, S], BF16)
            kT_sb = pers.tile([P, MB, S], BF16)
            v_sb = pers.tile([P, KT, HPC, HEAD_DIM + 1], BF16)
            atn_sb = pers.tile([P, MB, S], BF16)
            ones64 = constp.tile([1, HEAD_DIM], F32)
            nc.vector.memset(ones64[:], 1.0)

            # weight DMAs all ride the scalar queue in priority order; the
            # r=0 slices of wq/wk go first so the projection pipeline can
            # start as soon as ~128KB has landed. HBM BW is the startup
            # limiter, so order == priority.
            # critical r0 slices + wv/mask on the scalar queue (few
            # descriptors — exp activations follow on this queue), the bulk
            # on the otherwise-idle gpsimd ring
            nc.scalar.dma_start(wq_sb[:, 0, :], wq[0:P, :])
            nc.scalar.dma_start(wk_sb[:, 0, :], wk[0:P, :])
            # causal mask built on-device: mk[k, 512j+q] = 1 iff q >= 128j+k.
            # The affine_select goes FIRST on the gpsimd queue — behind the
            # bulk weight DMAs it would not be ready for chunk 0's first
            # diagonal mask-multiply (~19us in).
            nc.vector.memset(mk_sb[:], 1.0)
            nc.gpsimd.affine_select(
                mk_sb[:], mk_sb[:], pattern=[[-P, 4], [1, SCH]],
                compare_op=mybir.AluOpType.is_ge, fill=0.0,
                channel_multiplier=-1)
            for r in range(1, IT):
                nc.gpsimd.dma_start(wq_sb[:, r, :], wq[r * P:(r + 1) * P, :])
            for r in range(1, IT):
                nc.gpsimd.dma_start(wk_sb[:, r, :], wk[r * P:(r + 1) * P, :])
            for r in range(IT):
                nc.scalar.dma_start(wv_sb[:, r, :], wv[r * P:(r + 1) * P, :])
            nc.vector.memset(v_sb[:, :, :, HEAD_DIM:HEAD_DIM + 1], 1.0)

            _xc = {}

            def load_x(x_dram, c, name, split=False):
                # whole chunk c of one input tensor; the very first chunk
                # loads per-r so matmuls start when the first 128KB lands
                if name in _xc:
                    return _xc.pop(name)
                xt = inp.tile([P, IT, SCH], BF16, tag="inp", name=name)
                if split:
                    for r in range(IT):
                        nc.sync.dma_start(xt[:, r, :], x_dram[r, c])
                else:
                    nc.sync.dma_start(xt[:], x_dram[:, c].rearrange("r p s -> p r s"))
                return xt

            def qk_proj(c, parts=(0, 1), split=False):
                # Q and K projections for s-chunk c (part 0 = Q, 1 = K)
                srcs = ((xq, wq_sb, qT_sb, "xq"), (xk, wk_sb, kT_sb, "xk"))
                for part in parts:
                    x_dram, w_sb, dst, nm = srcs[part]
                    xt = load_x(x_dram, c, f"{nm}{c}", split=split)
                    ps = [psA.tile([P, SCH], F32, tag="psA", name=f"psqk{m}")
                          for m in range(MB)]
                    for r in range(IT):
                        for m in range(MB):
                            nc.tensor.matmul(
                                ps[m][:], w_sb[:, r, m * P:(m + 1) * P],
                                xt[:, r, :],
                                start=(r == 0), stop=(r == IT - 1))
                    for m in range(MB):
                        nc.vector.tensor_copy(dst[:, m, c * SCH:(c + 1) * SCH],
                                              ps[m][:])

            def v_proj(c, split=False):
                # V projection for s-tiles 4c..4c+3; j-pairs interleave so
                # consecutive matmuls hit different PSUM banks and pipeline
                xt = load_x(xv, c, f"xv{c}", split=split)
                for jp in (0, 2):
                    ps = [psA.tile([P, DLOC], F32, tag="psA", name=f"psv{jj}")
                          for jj in range(2)]
                    for r in range(IT):
                        for jj in range(2):
                            j = jp + jj
                            nc.tensor.matmul(
                                ps[jj][:], xt[:, r, j * P:(j + 1) * P],
                                wv_sb[:, r, :],
                                start=(r == 0), stop=(r == IT - 1))
                    for jj in range(2):
                        nc.vector.tensor_copy(
                            v_sb[:, 4 * c + jp + jj, :, 0:HEAD_DIM],
                            ps[jj][:].rearrange("p (h d) -> p h d", h=HPC))

            def _ps_pool(i, pools):
                # rotate Wo PSUM tiles across pools so the matmul stream is
                # not serialized behind each tile's PSUM->SBUF cast
                pool = pools[i % len(pools)]
                return pool.tile([P, SCH], F32, tag=pool.name, name="pso")

            def wo_proj(c, ts=(0, 1, 2, 3), pools=(psA, psO)):
                # output projection for s-tiles 4c+ts; stores alternate
                # between the gpsimd and sync rings (sync is input-only and
                # idle by the time stores begin) so the final flush halves
                i = 0
                for t in [4 * c + i for i in ts]:
                    for oc in range(D_MODEL // SCH):
                        ps_o = _ps_pool(i, pools)
                        for m in range(MB):
                            nc.tensor.matmul(
                                ps_o[:], atn_sb[:, m, t * P:(t + 1) * P],
                                wo_sb[:, m, oc * SCH:(oc + 1) * SCH],
                                start=(m == 0), stop=(m == MB - 1))
                        ot = ostage.tile([P, SCH], BF16, tag="ot")
                        nc.vector.tensor_copy(ot[:], ps_o[:])
                        ring = nc.gpsimd if i % 2 == 0 else nc.sync
                        ring.dma_start(
                            outp[t * P:(t + 1) * P, oc * SCH:(oc + 1) * SCH], ot[:])
                        i += 1

            def norm_head(h, c, ps_at, pe_bcast=False):
                # AT[0:64] *= broadcast(1/l);  l = ps_at row 64.
                # approx_fast mishandles partition-offset inputs: stage the
                # l row to partition 0 first. The broadcast normally rides
                # GpSimd (~1us, hidden mid-kernel); for the final chunk the
                # chain is exposed, so it runs as a tiny PE matmul instead.
                m, po = h // 2, (h % 2) * HEAD_DIM
                lrow = small.tile([1, SCH], F32, tag="lrow")
                nc.vector.tensor_copy(lrow[:], ps_at[HEAD_DIM:HEAD_DIM + 1, :])
                linv = small.tile([1, SCH], F32, tag="linv")
                nc.vector.reciprocal_approx_fast(out=linv[:], in_=lrow[:])
                dst = atn_sb[po:po + HEAD_DIM, m, c * SCH:(c + 1) * SCH]
                if pe_bcast:
                    # DVE reads at most one PSUM operand: stage AT to SBUF
                    # (overlaps the recip), broadcast on PE, multiply in place
                    nc.vector.tensor_copy(dst, ps_at[0:HEAD_DIM, :])
                    lbc = psA.tile([HEAD_DIM, SCH], F32, tag="psA", name="lbc")
                    nc.tensor.matmul(lbc[:], ones64[:], linv[:],
                                     start=True, stop=True)
                    nc.vector.tensor_mul(dst, dst, lbc[:])
                else:
                    lbc = small.tile([HEAD_DIM, SCH], F32, tag="lbc")
                    nc.gpsimd.partition_broadcast(lbc[:], linv[:])
                    nc.vector.tensor_mul(dst, ps_at[0:HEAD_DIM, :], lbc[:])

            qk_proj(0, split=True)
            v_proj(0)
            # wo isn't needed until the first wo_proj (~75us in): load it
            # after the startup-critical transfers instead of against them
            nc.gpsimd.dma_start(wo_sb[:], wo[:].rearrange("(m p) o -> p m o", p=P))

            # Chunks run 0,1,3,2: the exp-heaviest chunk (3) then still has
            # chunk-2 projections as dense PE filler, and the kernel tail is
            # the cheaper chunk 2 instead of 3. Chunk 3 needs kT/V of ALL
            # chunks first, so k/v of 2 complete during chunk 1.
            # Filler work is emitted between scores and AV of each
            # (chunk, head-pair): the in-order PE queue then has independent
            # work in front of the exp-gated AV chain while ScalarE catches
            # up. Slot sizing matches exp duration (~1us per k-tile pair).
            fillers = {
                (0, 0): [lambda: qk_proj(1)],
                (0, 1): [lambda: v_proj(1)],
                (1, 0): [lambda: qk_proj(3)],
                (1, 1): [lambda: qk_proj(2, parts=(1,)), lambda: v_proj(2)],
                (3, 0): [lambda: v_proj(3), lambda: wo_proj(0, ts=(0, 1))],
                (3, 1): [lambda: qk_proj(2, parts=(0,)),
                         lambda: wo_proj(0, ts=(2, 3))],
                (2, 0): [lambda: wo_proj(1)],
                (2, 1): [lambda: wo_proj(3)],
            }

            # ---- attention: chunk-major, head pairs, PE filler interleaved ----
            for c in (0, 1, 3, 2):
                nkt = 4 * (c + 1)  # causal: k-tiles 0..nkt-1

                def qoff(kt):
                    # diagonal k-tile j only needs q in [128j, 512)
                    return max(kt - 4 * c, 0) * P

                for hp in range(HPC // 2):
                    atts = []
                    for kt in range(nkt):
                        qo = qoff(kt)
                        ps_s = psS.tile([P, 2, SCH], F32, tag="psS")
                        for hh in range(2):
                            h = 2 * hp + hh
                            m, po = h // 2, (h % 2) * HEAD_DIM
                            nc.tensor.matmul(
                                ps_s[:, hh, qo:],
                                kT_sb[po:po + HEAD_DIM, m, kt * P:(kt + 1) * P],
                                qT_sb[po:po + HEAD_DIM, m,
                                      c * SCH + qo:(c + 1) * SCH],
                                start=True, stop=True)
                        att = attnp.tile([P, 2, SCH], BF16, tag="attn")
                        nc.scalar.activation(att[:, :, qo:],
                                             ps_s[:, :, qo:], Exp, scale=0.125)
                        j = kt - 4 * c
                        if j >= 0:  # diagonal tiles: apply causal mask
                            for hh in range(2):
                                nc.vector.tensor_mul(
                                    att[:, hh, qo:], att[:, hh, qo:],
                                    mk_sb[:, j * SCH + qo:(j + 1) * SCH])
                        atts.append(att)

                    for f in fillers.pop((c, hp), ()):
                        f()

                    # the two heads' AV chains interleave so consecutive
                    # matmuls hit different PSUM banks and pipeline (a
                    # single accumulation chain serializes on its bank)
                    ps_at = [psO.tile([HEAD_DIM + 1, SCH], F32, tag="psO",
                                      name=f"ps_at{hh}") for hh in range(2)]
                    for kt in range(nkt):
                        qo = qoff(kt)
                        for hh in range(2):
                            nc.tensor.matmul(
                                ps_at[hh][:, qo:], v_sb[:, kt, 2 * hp + hh, :],
                                atts[kt][:, hh, qo:],
                                start=(kt == 0), stop=(kt == nkt - 1))
                    for hh in range(2):
                        norm_head(2 * hp + hh, c, ps_at[hh],
                                  pe_bcast=(c == 2 and hp == 1))
            wo_proj(2, pools=(psA, psO, psS))

    nc.compile()
    return nc


def _get_nc():
    if "nc" not in _CACHE:
        _CACHE["nc"] = _build()
    return _CACHE["nc"]


def _tile_xt(x_t):
    # [D_MODEL, S] -> [IT, NCH, 128, 512] contiguous tiles
    return np.ascontiguousarray(
        x_t.reshape(IT, P, NCH, SCH).transpose(0, 2, 1, 3))


def _kernel_numpy(query, key, value, mask, Wq, Wk, Wv, Wo, bo):
    # exact f32 fallback for non-causal masks
    q = (query @ Wq.T).reshape(B, S, NUM_HEADS, HEAD_DIM).transpose(0, 2, 1, 3)
    k = (key @ Wk.T).reshape(B, S, NUM_HEADS, HEAD_DIM).transpose(0, 2, 1, 3)
    v = (value @ Wv.T).reshape(B, S, NUM_HEADS, HEAD_DIM).transpose(0, 2, 1, 3)
    s = np.einsum("bhqd,bhkd->bhqk", q, k) / np.sqrt(np.float32(HEAD_DIM))
    s = np.where(np.asarray(mask), s, -np.inf)
    s = s - s.max(axis=-1, keepdims=True)
    e = np.exp(s)
    a = e / e.sum(axis=-1, keepdims=True)
    o = np.einsum("bhqk,bhkd->bhqd", a, v).transpose(0, 2, 1, 3)
    return (o.reshape(B, S, D_MODEL) @ Wo.T + bo).astype(np.float32)


def kernel(query, key, value, mask, Wq, Wk, Wv, Wo, bo):
    from concourse.bass_utils import run_bass_kernel_spmd

    m = np.asarray(mask).astype(bool)
    expect = np.tril(np.ones((S, S), dtype=bool))
    if m.size != S * S or not np.array_equal(m.reshape(S, S), expect):
        args = [np.asarray(a, np.float32) for a in
                (query, key, value)] + [mask] + [
                np.asarray(a, np.float32) for a in (Wq, Wk, Wv, Wo, bo)]
        return _kernel_numpy(*args)

    nc = _get_nc()
    bf = ml_dtypes.bfloat16

    xq_t = [_tile_xt(np.asarray(query)[b].T.astype(bf)) for b in range(B)]
    xk_t = [_tile_xt(np.asarray(key)[b].T.astype(bf)) for b in range(B)]
    xv_t = [_tile_xt(np.asarray(value)[b].T.astype(bf)) for b in range(B)]
    WqT = np.ascontiguousarray(np.asarray(Wq).T).astype(bf)  # [D, D] cols = out dim
    WkT = np.ascontiguousarray(np.asarray(Wk).T).astype(bf)
    WvT = np.ascontiguousarray(np.asarray(Wv).T).astype(bf)
    WoT = np.ascontiguousarray(np.asarray(Wo).T).astype(bf)

    in_maps = []
    for core in range(N_CORES):
        b, g = core // GROUPS, core % GROUPS
        hsl = slice(g * DLOC, (g + 1) * DLOC)
        in_maps.append({
            "xq_t": xq_t[b], "xk_t": xk_t[b], "xv_t": xv_t[b],
            "wq_t": np.ascontiguousarray(WqT[:, hsl]),
            "wk_t": np.ascontiguousarray(WkT[:, hsl]),
            "wv_t": np.ascontiguousarray(WvT[:, hsl]),
            "wo_t": np.ascontiguousarray(WoT[hsl, :]),
        })

    res = run_bass_kernel_spmd(nc, in_maps, core_ids=list(range(N_CORES)))
    _CACHE["last_result"] = res

    out = np.zeros((B, S, D_MODEL), np.float32)
    for core in range(N_CORES):
        out[core // GROUPS] += np.asarray(res.results[core]["outp"],
                                          np.float32)
    out += np.asarray(bo, np.float32)[None, None, :]
    return out
